# revision 7
# baseline (speedup 1.0000x reference)
"""Trainium2 Bass kernel: VQ codebook lookup + intra-sample attention +
cross-sample NxN attention, sharded over 8 NeuronCores.

Strategy (per sharding hint): data-parallel over batch. Core c owns rows
[c*800, c*800+800) of the flattened [6400, 64] input (8 samples each).
Each core receives the full input transposed-and-rolled so that its own
rows sit at columns [0, 800) — this makes the program identical across
cores (true SPMD) while the cross-sample mask always lands on column
blocks 0..7. Q/V projections for the cross-sample attention are
recomputed per core (cheap) so no collectives are needed.

All biases are folded in via an ones-row appended to the transposed
input and bias rows appended to the weights (host-side augmentation).
Softmax is computed without max-subtraction (scores are O(5), exp is
safe in f32) using an appended ones-column on V to get the denominator
from the same matmul as the numerator.
"""

import numpy as np
from contextlib import ExitStack

import concourse.bass as bass
import concourse.tile as tile
from concourse import bacc, mybir
from concourse.bass_utils import run_bass_kernel_spmd
from concourse.masks import make_identity

F32 = mybir.dt.float32
AX = mybir.AxisListType
ALU = mybir.AluOpType
ACTF = mybir.ActivationFunctionType

BS, DN, SL, DIM = 64, 2, 50, 64
NE = 512                  # codebook size
N = BS * DN * SL          # 6400 flattened rows
NCORES = 8
PER = N // NCORES         # 800 rows per core
SAMP = DN * SL            # 100 rows per sample
NPAIR = PER // SL         # 16 (sample, domain) pairs per core
NSAMP = PER // SAMP       # 8 samples per core

# weight-pack column offsets (all share the 65-partition layout)
OFF_QC, OFF_KC, OFF_VC = 0, 64, 128          # wv is 65 wide (ones col)
OFF_QI, OFF_KI, OFF_VI = 193, 257, 321
OFF_CT2 = 386
WPW = OFF_CT2 + NE                            # 898

TRACE = False
TRACE_KWARGS = {}
LAST_RESULTS = None
_CACHE = {}


def _mask_table():
    """Per j-block (128 rows) multiplicative masks for the block-diagonal
    same-sample mask. Own rows are cols 0..799 (samples 0..7); j rows
    [128*jb, 128*jb+128) overlap samples s in [j0//100, (j0+127)//100].
    Returns [(jb, c0, w, rects)] where rects are (a, b, i0, i1) zero
    boxes relative to (0, c0)."""
    out = []
    for jb in range(7):
        j0 = jb * 128
        s_lo = j0 // SAMP
        s_hi = min((j0 + 127) // SAMP, NSAMP - 1)
        c0 = s_lo * SAMP
        w = (s_hi + 1) * SAMP - c0
        rects = []
        for s in range(s_lo, s_hi + 1):
            a = max(0, s * SAMP - j0)
            b = min(128, (s + 1) * SAMP - j0)
            rects.append((a, b, s * SAMP - c0, (s + 1) * SAMP - c0))
        out.append((jb, c0, w, rects))
    return out


MASKS = _mask_table()
MSK_W = sum(w for _, _, w, _ in MASKS)   # 1300
CBW = 4 * DIM + MSK_W                    # cb input: codebook chunks + masks


def _ceil_div(a, b):
    return -(-a // b)


def _emit(ctx, tc, xt_d, wp_d, cb_d, q_d, z_d, x_d):
    nc = tc.nc

    consts = ctx.enter_context(tc.tile_pool(name="consts", bufs=1))
    bigs = ctx.enter_context(tc.tile_pool(name="bigs", bufs=1))

    # ---- inputs -> SBUF ----
    HALF = N // 2
    xt0 = consts.tile([DIM + 1, HALF], F32, tag="xt0")
    xt1 = consts.tile([DIM + 1, HALF], F32, tag="xt1")
    nc.sync.dma_start(out=xt0, in_=xt_d[:, 0:HALF])
    nc.sync.dma_start(out=xt1, in_=xt_d[:, HALF:N])

    def xtc(off, width):
        """Slice [65, width] of the (split) transposed input."""
        if off + width <= HALF:
            return xt0[:, off:off + width]
        assert off >= HALF
        return xt1[:, off - HALF:off - HALF + width]

    wp = consts.tile([DIM + 1, WPW], F32, tag="wp")
    nc.sync.dma_start(out=wp, in_=wp_d)
    cbm = consts.tile([128, CBW], F32, tag="cbm")
    nc.sync.dma_start(out=cbm, in_=cb_d)
    cb = cbm[:, 0:4 * DIM]
    ident = consts.tile([128, 128], F32, tag="ident")
    make_identity(nc, ident)

    WQC = wp[:, OFF_QC:OFF_QC + 64]
    WKC = wp[:, OFF_KC:OFF_KC + 64]
    WVC = wp[:, OFF_VC:OFF_VC + 65]
    WQI = wp[:, OFF_QI:OFF_QI + 64]
    WKI = wp[:, OFF_KI:OFF_KI + 64]
    WVI = wp[:, OFF_VI:OFF_VI + 65]
    CT2 = wp[:, OFF_CT2:OFF_CT2 + NE]

    # ---- persistent SBUF intermediates ----
    qcT = bigs.tile([DIM, N], F32, tag="qcT")          # cs Q^T, all rows
    kcT = bigs.tile([DIM, PER], F32, tag="kcT")        # cs K^T, own rows
    qiT = bigs.tile([DIM, PER], F32, tag="qiT")        # is Q^T, own rows
    kiT = bigs.tile([DIM, PER], F32, tag="kiT")        # is K^T, own rows
    vcaug = bigs.tile([128, 50 * 65], F32, tag="vcaug")  # cs V rows + ones col

    # ================= projections =================
    PJ = 400  # chunk width; divides the 3200 halves evenly
    with tc.tile_pool(name="pp", bufs=2, space="PSUM") as pp, \
         tc.tile_pool(name="po", bufs=2, space="PSUM") as po, \
         tc.tile_pool(name="vg", bufs=2, space="PSUM") as vg:

        # qcT over all 6400 columns, 400 at a time
        for k in range(N // PJ):
            ps = pp.tile([DIM, PJ], F32, tag="ps")
            nc.tensor.matmul(ps, WQC, xtc(k * PJ, PJ), start=True, stop=True)
            nc.any.tensor_copy(qcT[:, k * PJ:(k + 1) * PJ], ps)

        # own-row projections (kcT, qiT, kiT): two <=512 matmuls into a
        # 2-bank psum tile, one copy out
        for dst, w in ((kcT, WKC), (qiT, WQI), (kiT, WKI)):
            pt = po.tile([DIM, PER], F32, tag="po")
            nc.tensor.matmul(pt[:, 0:512], w, xtc(0, 512), start=True, stop=True)
            nc.tensor.matmul(pt[:, 512:PER], w, xtc(512, PER - 512),
                             start=True, stop=True)
            nc.any.tensor_copy(dst, pt)

        # cs V rows (+bias +ones col) directly from x^T: groups of 7 jb
        for g in range(_ceil_div(50, 7)):
            nj = min(7, 50 - g * 7)
            vt = vg.tile([128, 7, 65], F32, tag="vg")
            for j in range(nj):
                jb = g * 7 + j
                nc.tensor.matmul(vt[:, j, :], xtc(jb * 128, 128), WVC,
                                 start=True, stop=True)
            nc.any.tensor_copy(
                vcaug[:, g * 7 * 65:(g * 7 + nj) * 65], vt[:, 0:nj, :])

    # ================= cross-sample attention =================
    # PSUM budget (8 banks): st 2x2 + ut 1x2 + smallp 2x1 = 8
    csp = ctx.enter_context(tc.tile_pool(name="csp", bufs=2, space="PSUM"))
    utp = ctx.enter_context(tc.tile_pool(name="utp", bufs=1, space="PSUM"))
    smallp = ctx.enter_context(tc.tile_pool(name="smallp", bufs=2, space="PSUM"))
    css = ctx.enter_context(tc.tile_pool(name="css", bufs=2))
    cse = ctx.enter_context(tc.tile_pool(name="cse", bufs=2))

    _n_small = [0]

    def small_tile(shape):
        # all epilogue/VQ/IS psum tiles are <= 1 bank; rotate 2 slots
        _n_small[0] += 1
        return smallp.tile(shape, F32, tag="small",
                           name=f"small{_n_small[0]}")

    ut = utp.tile([65, PER], F32, tag="ut")  # [aug_e, own_i] accumulator
    for jb in range(50):
        st = csp.tile([128, PER], F32, tag="st")
        nc.tensor.matmul(st[:, 0:512], qcT[:, jb * 128:(jb + 1) * 128],
                         kcT[:, 0:512], start=True, stop=True)
        nc.tensor.matmul(st[:, 512:PER], qcT[:, jb * 128:(jb + 1) * 128],
                         kcT[:, 512:PER], start=True, stop=True)
        est = css.tile([128, PER], F32, tag="est")
        nc.scalar.activation(est, st, ACTF.Exp)
        # mask: zero exp(score) where col-sample == row-sample (own rows
        # are global cols 0..799 after the per-core roll). Partition
        # starts must be 32-aligned, so use host-built 0/1 masks.
        if jb < 7:
            _, c0, w, _ = MASKS[jb]
            moff = 4 * DIM + sum(m[2] for m in MASKS[:jb])
            nc.vector.tensor_mul(est[:, c0:c0 + w], est[:, c0:c0 + w],
                                 cbm[:, moff:moff + w])
        nc.tensor.matmul(ut[:, 0:512], vcaug[:, jb * 65:jb * 65 + 65],
                         est[:, 0:512], start=(jb == 0), stop=(jb == 49),
                         skip_group_check=True)
        nc.tensor.matmul(ut[:, 512:PER], vcaug[:, jb * 65:jb * 65 + 65],
                         est[:, 512:PER], start=(jb == 0), stop=(jb == 49),
                         skip_group_check=True)

    ut_s = cse.tile([65, PER], F32, tag="ut_s")
    nc.any.tensor_copy(ut_s, ut)
    for g in range(2):
        xp = small_tile([100, 4, 65])
        for k in range(4):
            s = g * 4 + k
            nc.tensor.transpose(xp[:, k, :], ut_s[:, s * SAMP:(s + 1) * SAMP],
                                ident[0:65, 0:65])
        dr = cse.tile([100, 4], F32, tag="dr")
        nc.vector.reciprocal(dr, xp[:, :, 64])
        xg = cse.tile([100, 4, DIM], F32, tag="xg")
        for k in range(4):
            nc.vector.tensor_scalar_mul(xg[:, k, :], xp[:, k, 0:DIM],
                                        dr[:, k:k + 1])
        nc.sync.dma_start(
            out=x_d[g * 400:(g + 1) * 400, :].rearrange(
                "(s p) e -> p s e", p=SAMP),
            in_=xg)

    # ============ VQ + intra-sample attention (fill idle engines) ============
    vqs = ctx.enter_context(tc.tile_pool(name="vqs", bufs=2))

    # ---- VQ codebook lookup over own rows ----
    nchunk = _ceil_div(PER, 128)
    for k in range(nchunk):
        co = k * 128
        cw = min(128, PER - co)
        dps = small_tile([128, NE])
        nc.tensor.matmul(dps[0:cw, :], xtc(co, cw), CT2, start=True, stop=True)
        minv = vqs.tile([128, 1], F32, tag="minv")
        nc.vector.tensor_reduce(out=minv[0:cw, :], in_=dps[0:cw, :],
                                axis=AX.X, op=ALU.min)
        oh = vqs.tile([128, NE], F32, tag="oh")
        nc.vector.tensor_scalar(out=oh[0:cw, :], in0=dps[0:cw, :],
                                scalar1=minv[0:cw, :], scalar2=None,
                                op0=ALU.is_equal)
        cnt = vqs.tile([128, 1], F32, tag="cnt")
        nc.vector.reduce_sum(out=cnt[0:cw, :], in_=oh[0:cw, :], axis=AX.X)
        rcnt = vqs.tile([128, 1], F32, tag="rcnt")
        nc.vector.reciprocal(rcnt[0:cw, :], cnt[0:cw, :])
        tr = small_tile([128, 4, 128])
        for cc in range(4):
            nc.tensor.transpose(tr[:, cc, 0:cw],
                                oh[0:cw, cc * 128:(cc + 1) * 128],
                                ident[0:cw, 0:cw])
        oht = vqs.tile([128, 4, 128], F32, tag="oht")
        nc.vector.tensor_copy(oht[:, :, 0:cw], tr[:, :, 0:cw])
        qps = small_tile([128, DIM])
        for cc in range(4):
            nc.tensor.matmul(qps[0:cw, :], oht[:, cc, 0:cw],
                             cb[:, cc * DIM:(cc + 1) * DIM],
                             start=(cc == 0), stop=(cc == 3),
                             skip_group_check=True)
        qs = vqs.tile([128, DIM], F32, tag="qs")
        nc.vector.tensor_scalar_mul(qs[0:cw, :], qps[0:cw, :], rcnt[0:cw, :])
        nc.sync.dma_start(out=q_d[co:co + cw, :], in_=qs[0:cw, :])

    # ---- intra-sample attention: 16 independent 50x50 attentions ----
    iss = ctx.enter_context(tc.tile_pool(name="iss", bufs=2))
    isb = ctx.enter_context(tc.tile_pool(name="isb", bufs=1))

    vaug_s = isb.tile([SL, NPAIR * 65], F32, tag="vaug_s")
    for g in range(_ceil_div(NPAIR, 7)):
        np_ = min(7, NPAIR - g * 7)
        vp = small_tile([SL, 7, 65])
        for j in range(np_):
            p = g * 7 + j
            nc.tensor.matmul(vp[:, j, :], xtc(p * SL, SL), WVI,
                             start=True, stop=True)
        nc.any.tensor_copy(vaug_s[:, g * 7 * 65:(g * 7 + np_) * 65],
                           vp[:, 0:np_, :])

    est_is = isb.tile([SL, NPAIR * SL], F32, tag="est_is")
    for h in range(2):
        stt = small_tile([SL, 8, 64])
        for j in range(8):
            p = h * 8 + j
            nc.tensor.matmul(stt[:, j, 0:SL], qiT[:, p * SL:(p + 1) * SL],
                             kiT[:, p * SL:(p + 1) * SL],
                             start=True, stop=True)
        nc.scalar.activation(est_is[:, h * 8 * SL:(h + 1) * 8 * SL],
                             stt[:, :, 0:SL], ACTF.Exp)

    z_s = isb.tile([SL, NPAIR, DIM], F32, tag="z_s")
    for g in range(_ceil_div(NPAIR, 7)):
        np_ = min(7, NPAIR - g * 7)
        zz = small_tile([SL, 7, 65])
        for j in range(np_):
            p = g * 7 + j
            nc.tensor.matmul(zz[:, j, :], est_is[:, p * SL:(p + 1) * SL],
                             vaug_s[:, p * 65:p * 65 + 65],
                             start=True, stop=True)
        drz = iss.tile([SL, 7], F32, tag="drz")
        nc.vector.reciprocal(drz[:, 0:np_], zz[:, 0:np_, 64])
        for j in range(np_):
            p = g * 7 + j
            nc.vector.tensor_scalar_mul(z_s[:, p, :], zz[:, j, 0:DIM],
                                        drz[:, j:j + 1])
    nc.sync.dma_start(
        out=z_d.rearrange("(q t) e -> t q e", t=SL), in_=z_s)


def _build():
    nc = bacc.Bacc("TRN2", target_bir_lowering=False, debug=False,
                   num_devices=NCORES)
    xt_d = nc.dram_tensor("xt", [DIM + 1, N], F32, kind="ExternalInput").ap()
    wp_d = nc.dram_tensor("wp", [DIM + 1, WPW], F32, kind="ExternalInput").ap()
    cb_d = nc.dram_tensor("cb", [128, CBW], F32, kind="ExternalInput").ap()
    q_d = nc.dram_tensor("q_out", [PER, DIM], F32, kind="ExternalOutput").ap()
    z_d = nc.dram_tensor("z_out", [PER, DIM], F32, kind="ExternalOutput").ap()
    x_d = nc.dram_tensor("x_out", [PER, DIM], F32, kind="ExternalOutput").ap()

    with tile.TileContext(nc) as tc:
        with ExitStack() as ctx:
            _emit(ctx, tc, xt_d, wp_d, cb_d, q_d, z_d, x_d)
    nc.compile()
    return nc


def _host_inputs(x, code_book,
                 Wq_is, bq_is, Wk_is, bk_is, Wv_is, bv_is,
                 Wq_cs, bq_cs, Wk_cs, bk_cs, Wv_cs, bv_cs):
    f = np.float32
    flatT = np.ascontiguousarray(
        np.asarray(x, f).reshape(N, DIM).T)          # [64, 6400]

    def waug(W, b):                                   # [65, 64]
        return np.concatenate(
            [np.asarray(W, f), np.asarray(b, f).reshape(1, DIM)], axis=0)

    def waug_ones(W, b):                              # [65, 65]
        out = np.zeros((DIM + 1, DIM + 1), f)
        out[:DIM, :DIM] = np.asarray(W, f)
        out[DIM, :DIM] = np.asarray(b, f)
        out[DIM, DIM] = 1.0
        return out

    C = np.asarray(code_book, f)
    wp = np.zeros((DIM + 1, WPW), f)
    wp[:, OFF_QC:OFF_QC + 64] = waug(Wq_cs, bq_cs)
    wp[:, OFF_KC:OFF_KC + 64] = waug(Wk_cs, bk_cs)
    wp[:, OFF_VC:OFF_VC + 65] = waug_ones(Wv_cs, bv_cs)
    wp[:, OFF_QI:OFF_QI + 64] = waug(Wq_is, bq_is)
    wp[:, OFF_KI:OFF_KI + 64] = waug(Wk_is, bk_is)
    wp[:, OFF_VI:OFF_VI + 65] = waug_ones(Wv_is, bv_is)
    wp[0:DIM, OFF_CT2:] = -2.0 * C.T
    wp[DIM, OFF_CT2:] = (C * C).sum(axis=1)

    mblocks = []
    for _, c0, w, rects in MASKS:
        m = np.ones((128, w), f)
        for a, b, i0, i1 in rects:
            m[a:b, i0:i1] = 0.0
        mblocks.append(m)
    cbp = np.concatenate(
        [C[cc * 128:(cc + 1) * 128] for cc in range(4)] + mblocks,
        axis=1)                                       # [128, 256 + 1300]

    ones = np.ones((1, N), f)
    in_maps = []
    for c in range(NCORES):
        xt = np.concatenate([np.roll(flatT, -c * PER, axis=1), ones], axis=0)
        in_maps.append({"xt": np.ascontiguousarray(xt), "wp": wp, "cb": cbp})
    return in_maps


def kernel(**inputs):
    global LAST_RESULTS
    if "nc" not in _CACHE:
        _CACHE["nc"] = _build()
    nc = _CACHE["nc"]

    in_maps = _host_inputs(**inputs)
    res = run_bass_kernel_spmd(nc, in_maps, list(range(NCORES)),
                               trace=TRACE, trace_kwargs=TRACE_KWARGS)
    LAST_RESULTS = res
    outs = res.results
    shape = (BS, DN, SL, DIM)
    quant = np.concatenate([outs[c]["q_out"] for c in range(NCORES)], axis=0)
    z = np.concatenate([outs[c]["z_out"] for c in range(NCORES)], axis=0)
    x = np.concatenate([outs[c]["x_out"] for c in range(NCORES)], axis=0)
    return (quant.reshape(shape).astype(np.float32),
            z.reshape(shape).astype(np.float32),
            x.reshape(shape).astype(np.float32))
